# revision 4
# baseline (speedup 1.0000x reference)
"""Trainium2 Bass kernel for nn_AxonalConnections (gnn_message_passing).

Computes, for 4 modules with 12 directed pairs (s, d), s != d:
    out[d] = sum_{s != d} x[s] @ W[(s,d)].T
             + strength[d] * (sin(t*local_freq[d]) + sin(t*global_freq[d]))
with x: [4, 2048, 1024] f32, W: [12, 1024, 1024] f32, t = 2*pi*clk*1e-3.

Sharding over 8 NeuronCores: core c = 2*d + h handles destination module d
and batch half h (1024 rows).  Per core: 3 GEMMs [1024,1024]@[1024,1024]
accumulated in PSUM.

Perf design (v2, bf16):
- Operands stream as bf16 (host-side round-to-nearest): halves HBM traffic
  vs fp32/f32r so DMA (~12 MiB in, ~40 us) sits well under the PE floor
  (384 matmuls x 512 cols x 1 cyc/row @ 2.4 GHz = 82 us).  Output stays f32.
- Transposed PSUM orientation: stationary = W tile [128K, 128 out-cols],
  moving = x tile [128K, N batch]; psum = [128 o, N b].  The oscillator
  bias then varies along PARTITIONS, so it is fused into the PSUM->SBUF
  eviction as a per-partition bias on the (otherwise idle) Activation and
  Vector engines - zero PE cycles spent on bias (the fp32r version burned
  16 K=1 matmuls = 3.4 us on it).
- Full output needs 16 psum banks (8 o-tiles x 1024 batch); only 8 exist.
  Three K-sweep passes over batch columns (512/448/64): each pass holds 8
  psum banks, sweeps all 24 (j,k) tiles in DMA arrival order.  Per step
  pass A consumes 384 KiB (W tile + x half) in 1.71 us of PE time while
  DMA delivers it in ~1.27 us, so the PE never starves.  The last pass is
  tiny (N=64) so the un-overlappable tail (final evictions + out DMA) is
  ~1 us of transfer instead of 7 us.
- One DMA launch costs ~665 ns on the issuing sequencer (the fp32r version
  spent 76 us of SP time on 114 launches).  This version issues ~77: bias,
  24 W + 24 xA interleaved in consumption order, 24 xB, 1 coalesced xC,
  and 3 coalesced out launches (one per pass, via a [128, o, b] dram
  layout that matches SBUF stream order; host un-permutes).
- Dummy warm-up matmuls during the DMA prologue hold the PE's HAM
  clock-gate at 2.4 GHz so real matmuls never run at the cold 0.65/1.2 GHz.
- The Bass program is built by code exec'd under a fixed pseudo-filename
  so the BIR (which embeds source debug locations) is byte-identical no
  matter where kernel.py lives - keeping the NEFF compile cache warm
  across directories.

Host-side prep is limited to slicing/transposing/rounding inputs into the
per-core layouts (contraction dim on partitions) and evaluating the tiny
[4,1024] oscillator bias (pure function of the small freq/strength/clk
inputs).
"""

import math
import sys
import threading

import ml_dtypes
import numpy as np

sys.path.insert(0, "/opt/trn_rl_repo")

from concourse.bass_utils import run_bass_kernel_spmd  # noqa: E402

N_MOD = 4
B = 2048
D = 1024
BH = B // 2  # batch rows per core
N_CORES = 8

PAIRS = [(s, d) for s in range(N_MOD) for d in range(N_MOD) if s != d]
PAIR_IDX = {sd: i for i, sd in enumerate(PAIRS)}
SRCS_OF = {d: [s for s in range(N_MOD) if s != d] for d in range(N_MOD)}

# batch-column pass sizes (see module docstring); last one small -> tiny tail
PASSES = (512, 448, 64)
POFF = (0, 512, 960)

_CACHED = {}

_BUILDER_FILENAME = "/bass_axonal_connections/builder_v3.py"
_BUILDER_SRC = '''
import concourse.mybir as mybir
from concourse import bacc
from concourse.tile import TileContext

D = 1024
F32 = mybir.dt.float32
BF16 = mybir.dt.bfloat16
NJ = 3            # sources per destination
KT = 8            # 128-row contraction tiles per source
NO = 8            # 128-col output tiles
N_WARM = 16
# k-tile chunking of the W / xA DMA streams: fine at the start (fast PE
# launch), coarsening once the pipeline is ahead (fewer launches + sems)
CHUNKS = {0: (1, 1, 2, 4), 1: (4, 4), 2: (4, 4)}

Identity = mybir.ActivationFunctionType.Identity


def build_nc():
    nc = bacc.Bacc(None, target_bir_lowering=False, debug=False)
    # all inputs partition-major, flat free dim:
    #   wt[j, p, k*1024 + o]  xa[j, p, k*512 + c]  (batch cols 0:512)
    #   xb[j, p, k*448 + c]   (cols 512:960)
    #   xc[p, (j*KT+k)*64+c]  (cols 960:1024)
    wt = nc.declare_dram_parameter("wt", [NJ, 128, KT * D], BF16, isOutput=False)
    xa = nc.declare_dram_parameter("xa", [NJ, 128, KT * 512], BF16, isOutput=False)
    xb = nc.declare_dram_parameter("xb", [NJ, 128, KT * 448], BF16, isOutput=False)
    xc = nc.declare_dram_parameter("xc", [128, NJ * KT * 64], BF16, isOutput=False)
    bias = nc.declare_dram_parameter("bias", [128, NO], F32, isOutput=False)
    # out[p, o_t, b] = outT[o_t*128+p, b]; host un-permutes
    out = nc.declare_dram_parameter("out", [128, NO, D], F32, isOutput=True)

    with TileContext(nc) as tc:
        with (
            tc.tile_pool(name="wpool", bufs=8) as wpool,
            tc.tile_pool(name="xpool", bufs=8) as xpool,
            tc.tile_pool(name="opool", bufs=1) as opool,
            tc.tile_pool(name="cpool", bufs=1) as cpool,
            tc.tile_pool(name="pspool", bufs=8, space="PSUM") as pspool,
        ):
            # PE warm-up: dummy matmuls during the DMA prologue ramp the
            # HAM p-state so real matmuls start at 2.4 GHz; sized to end
            # just as the first W/xA tiles land (~11 us)
            ones = cpool.tile([1, 128], BF16, tag="ones", name="ones")
            nc.vector.memset(ones, 1.0)
            warm = cpool.tile([1, 256], BF16, tag="warm", name="warm")
            nc.vector.memset(warm, 0.0)
            ps_warm = pspool.tile([128, 512], F32, tag="ps", name="ps_warm")
            for wi in range(N_WARM):
                nc.tensor.matmul(
                    ps_warm[:, 0:256], lhsT=ones, rhs=warm,
                    start=(wi == 0), stop=(wi == N_WARM - 1),
                )

            # input DMAs on SP in consumption order; W/xA interleaved per
            # k-chunk, bias tucked after the first pair (needed only at the
            # first eviction)
            wch = {}   # (j, chunk_start) -> (tile, nk)
            xach = {}
            first = True
            for j in range(NJ):
                k0 = 0
                for nk in CHUNKS[j]:
                    wti = wpool.tile([128, nk * D], BF16, tag="wt",
                                     name=f"wt_{j}_{k0}")
                    nc.sync.dma_start(
                        out=wti, in_=wt[j][:, k0 * D : (k0 + nk) * D]
                    )
                    xti = xpool.tile([128, nk * 512], BF16, tag="xa",
                                     name=f"xa_{j}_{k0}")
                    nc.sync.dma_start(
                        out=xti, in_=xa[j][:, k0 * 512 : (k0 + nk) * 512]
                    )
                    for kk in range(nk):
                        wch[j, k0 + kk] = (wti, kk)
                        xach[j, k0 + kk] = (xti, kk)
                    if first:
                        bias_sb = cpool.tile([128, NO], F32, tag="bias",
                                             name="bias_sb")
                        nc.sync.dma_start(out=bias_sb, in_=bias[:, :])
                        first = False
                    k0 += nk
            xbt = {}
            for j in range(NJ):
                xti = xpool.tile([128, KT * 448], BF16, tag="xb", name=f"xb_{j}")
                nc.sync.dma_start(out=xti, in_=xb[j])
                xbt[j] = xti
            xc_sb = cpool.tile([128, NJ * KT * 64], BF16, tag="xc", name="xc_sb")
            nc.sync.dma_start(out=xc_sb, in_=xc[:, :])

            jks = [(j, k) for j in range(NJ) for k in range(KT)]

            def evict(ot, psums, olist, npc, engs):
                # fused per-partition bias add on Act/DVE (alternating) so
                # psum banks free at ~matmul rate for the next pass
                for i, o in enumerate(olist):
                    dst = ot[:, i * npc : (i + 1) * npc]
                    src = psums[o][:, 0:npc]
                    bcol = bias_sb[:, o : o + 1]
                    if engs[i % len(engs)] == "act":
                        nc.scalar.activation(dst, src, Identity, bias=bcol)
                    else:
                        nc.vector.tensor_scalar_add(dst, src, bcol)

            # ---- pass A: batch cols 0:512, all 8 psum banks ----
            psA = [pspool.tile([128, 512], F32, tag="ps", name=f"psA_{o}")
                   for o in range(NO)]
            for step, (j, k) in enumerate(jks):
                wti, wkk = wch[j, k]
                xti, xkk = xach[j, k]
                rhs = xti[:, xkk * 512 : (xkk + 1) * 512]
                for o in range(NO):
                    nc.tensor.matmul(
                        psA[o],
                        lhsT=wti[:, wkk * D + o * 128 : wkk * D + (o + 1) * 128],
                        rhs=rhs,
                        start=(step == 0), stop=(step == len(jks) - 1),
                    )
            otA = opool.tile([128, NO * 512], F32, tag="otA", name="ot_A")
            evict(otA, psA, list(range(NO)), 512, ("act", "dve"))
            nc.sync.dma_start(
                out=out[:, :, 0:512],
                in_=otA.rearrange("p (o c) -> p o c", o=NO),
            )

            # ---- passes B (cols 512:960) and C (cols 960:1024), split into
            # half-width 4-psum sweeps so each half's out DMA overlaps the
            # next sweep and the final one is tiny ----
            for p_i, (npc, off, src_kind) in enumerate(
                ((448, 512, "xb"), (64, 960, "xc"))
            ):
                for half in range(2):
                    olist = list(range(half * 4, half * 4 + 4))
                    ps = [None] * NO
                    for o in olist:
                        ps[o] = pspool.tile([128, 512], F32, tag="ps",
                                            name=f"ps{p_i}_{half}_{o}")
                    for step, (j, k) in enumerate(jks):
                        wti, wkk = wch[j, k]
                        if src_kind == "xb":
                            rhs = xbt[j][:, k * 448 : (k + 1) * 448]
                        else:
                            jk64 = (j * KT + k) * 64
                            rhs = xc_sb[:, jk64 : jk64 + 64]
                        for o in olist:
                            nc.tensor.matmul(
                                ps[o][:, 0:npc],
                                lhsT=wti[:, wkk * D + o * 128 : wkk * D + (o + 1) * 128],
                                rhs=rhs,
                                start=(step == 0), stop=(step == len(jks) - 1),
                            )
                    ot = opool.tile([128, 4 * npc], F32, tag=f"ot{p_i}{half}",
                                    name=f"ot_{p_i}_{half}")
                    evict(ot, ps, olist, npc, ("act", "dve"))
                    nc.sync.dma_start(
                        out=out[:, half * 4 : half * 4 + 4, off : off + npc],
                        in_=ot.rearrange("p (o c) -> p o c", o=4),
                    )
    nc.finalize()
    return nc


def build_into(result):
    result["nc"] = build_nc()
'''

_builder_ns = {}
exec(compile(_BUILDER_SRC, _BUILDER_FILENAME, "exec"), _builder_ns)


def build_nc():
    """Build the (shared, SPMD) Bass program once.

    Runs in a thread whose entry point is the exec'd builder, so no frame
    with kernel.py's (location-dependent) path is on the stack while
    instructions capture debug info — the BIR stays byte-identical across
    directories and the NEFF compile cache stays warm."""
    result = {}
    t = threading.Thread(target=_builder_ns["build_into"], args=(result,))
    t.start()
    t.join()
    if "nc" not in result:
        # builder raised inside the thread; rebuild inline for a real trace
        return _builder_ns["build_nc"]()
    return result["nc"]


def make_in_maps(x, W, local_freq, global_freq, strength, current_clk):
    x = np.asarray(x, dtype=np.float32)
    W = np.asarray(W, dtype=np.float32)
    local_freq = np.asarray(local_freq, dtype=np.float32)
    global_freq = np.asarray(global_freq, dtype=np.float32)
    strength = np.asarray(strength, dtype=np.float32)
    clk = float(np.asarray(current_clk))
    t = 2.0 * math.pi * clk * 0.001

    bf16 = ml_dtypes.bfloat16
    in_maps = []
    for d in range(N_MOD):
        srcs = SRCS_OF[d]
        # wt[j]: [1024 src(K), 1024 dst(o)] = W[(s_j,d)].T, then
        # partition-major flat: [3, 128, k*1024 + o]
        wt_d = np.ascontiguousarray(
            np.stack([W[PAIR_IDX[(s, d)]].T for s in srcs])
            .astype(bf16)
            .reshape(3, 8, 128, D)
            .transpose(0, 2, 1, 3)
            .reshape(3, 128, 8 * D)
        )
        # host-evaluated oscillator bias, partition-major [128, 8]
        bias_row = strength[d] * (
            np.sin(t * local_freq[d]) + np.sin(t * global_freq[d])
        )
        bias_d = np.ascontiguousarray(
            bias_row.astype(np.float32).reshape(8, 128).T
        )
        for h in range(2):
            # xt[j]: [1024 K, 1024 b] = x[s_j, half].T, tiled [3,8,128,1024]
            xt_c = np.stack(
                [x[s, h * BH : (h + 1) * BH, :].T for s in srcs]
            ).astype(bf16).reshape(3, 8, 128, D)
            # batch-column pass slabs, partition-major flat
            xa_c = np.ascontiguousarray(
                xt_c[:, :, :, 0:512].transpose(0, 2, 1, 3).reshape(3, 128, 8 * 512)
            )
            xb_c = np.ascontiguousarray(
                xt_c[:, :, :, 512:960].transpose(0, 2, 1, 3).reshape(3, 128, 8 * 448)
            )
            xc_c = np.ascontiguousarray(
                xt_c[:, :, :, 960:].transpose(2, 0, 1, 3).reshape(128, 3 * 8 * 64)
            )
            in_maps.append(
                {"xa": xa_c, "xb": xb_c, "xc": xc_c, "wt": wt_d, "bias": bias_d}
            )
    return in_maps


def run(in_maps, trace=False, **kwargs):
    if "nc" not in _CACHED:
        _CACHED["nc"] = build_nc()
    res = run_bass_kernel_spmd(
        _CACHED["nc"], in_maps, core_ids=list(range(N_CORES)), trace=trace, **kwargs
    )
    return res


def kernel(x, W, local_freq, global_freq, strength, current_clk):
    in_maps = make_in_maps(x, W, local_freq, global_freq, strength, current_clk)
    res = run(in_maps)
    out = np.empty((N_MOD, B, D), dtype=np.float32)
    for d in range(N_MOD):
        for h in range(2):
            # res out[p, o_t, b] -> outT[o_t*128+p, b] -> [b, o]
            o_pb = res.results[2 * d + h]["out"]
            outT = o_pb.transpose(1, 0, 2).reshape(D, BH)
            out[d, h * BH : (h + 1) * BH, :] = outT.T
    return out


# revision 12
# speedup vs baseline: 1.0145x; 1.0145x over previous
"""Trainium2 Bass kernel for nn_AxonalConnections (gnn_message_passing).

Computes, for 4 modules with 12 directed pairs (s, d), s != d:
    out[d] = sum_{s != d} x[s] @ W[(s,d)].T
             + strength[d] * (sin(t*local_freq[d]) + sin(t*global_freq[d]))
with x: [4, 2048, 1024] f32, W: [12, 1024, 1024] f32, t = 2*pi*clk*1e-3.

Sharding over 8 NeuronCores: core c = 2*d + h handles destination module d
and batch half h (1024 rows).  Per core: 3 GEMMs [1024,1024]@[1024,1024]
accumulated in PSUM.

Perf design (v2, bf16):
- Operands stream as bf16 (host-side round-to-nearest): halves HBM traffic
  vs fp32/f32r so DMA (~12 MiB in, ~40 us) sits well under the PE floor
  (384 matmuls x 512 cols x 1 cyc/row @ 2.4 GHz = 82 us).  Output stays f32.
- Transposed PSUM orientation: stationary = W tile [128K, 128 out-cols],
  moving = x tile [128K, N batch]; psum = [128 o, N b].  The oscillator
  bias then varies along PARTITIONS, so it is fused into the PSUM->SBUF
  eviction as a per-partition bias on the (otherwise idle) Activation and
  Vector engines - zero PE cycles spent on bias (the fp32r version burned
  16 K=1 matmuls = 3.4 us on it).
- Full output needs 16 psum banks (8 o-tiles x 1024 batch); only 8 exist.
  Three K-sweep passes over batch columns (512/448/64): each pass holds 8
  psum banks, sweeps all 24 (j,k) tiles in DMA arrival order.  Per step
  pass A consumes 384 KiB (W tile + x half) in 1.71 us of PE time while
  DMA delivers it in ~1.27 us, so the PE never starves.  The last pass is
  tiny (N=64) so the un-overlappable tail (final evictions + out DMA) is
  ~1 us of transfer instead of 7 us.
- One DMA launch costs ~665 ns on the issuing sequencer (the fp32r version
  spent 76 us of SP time on 114 launches).  This version issues ~77: bias,
  24 W + 24 xA interleaved in consumption order, 24 xB, 1 coalesced xC,
  and 3 coalesced out launches (one per pass, via a [128, o, b] dram
  layout that matches SBUF stream order; host un-permutes).
- Dummy warm-up matmuls during the DMA prologue hold the PE's HAM
  clock-gate at 2.4 GHz so real matmuls never run at the cold 0.65/1.2 GHz.
- The Bass program is built by code exec'd under a fixed pseudo-filename
  so the BIR (which embeds source debug locations) is byte-identical no
  matter where kernel.py lives - keeping the NEFF compile cache warm
  across directories.

Host-side prep is limited to slicing/transposing/rounding inputs into the
per-core layouts (contraction dim on partitions) and evaluating the tiny
[4,1024] oscillator bias (pure function of the small freq/strength/clk
inputs).
"""

import math
import sys
import threading

import ml_dtypes
import numpy as np

sys.path.insert(0, "/opt/trn_rl_repo")

from concourse.bass_utils import run_bass_kernel_spmd  # noqa: E402

N_MOD = 4
B = 2048
D = 1024
BH = B // 2  # batch rows per core
N_CORES = 8

PAIRS = [(s, d) for s in range(N_MOD) for d in range(N_MOD) if s != d]
PAIR_IDX = {sd: i for i, sd in enumerate(PAIRS)}
SRCS_OF = {d: [s for s in range(N_MOD) if s != d] for d in range(N_MOD)}

# batch-column pass sizes (see module docstring); last one small -> tiny tail
PASSES = (512, 448, 64)
POFF = (0, 512, 960)

_CACHED = {}

_BUILDER_FILENAME = "/bass_axonal_connections/builder_v4.py"
_BUILDER_SRC = '''
import concourse.mybir as mybir
from concourse import bacc
from concourse.tile import TileContext

D = 1024
F32 = mybir.dt.float32
BF16 = mybir.dt.bfloat16
NJ = 3            # sources per destination
KT = 8            # 128-row contraction tiles per source
NO = 8            # 128-col output tiles
N_WARM = 18

Identity = mybir.ActivationFunctionType.Identity


def build_nc():
    nc = bacc.Bacc(None, target_bir_lowering=False, debug=False)
    # all inputs partition-major, flat free dim:
    #   wt[j, p, k*1024 + o]  xa[j, p, k*512 + c]  (batch cols 0:512)
    #   xb[j, p, k*448 + c]   (cols 512:960)
    #   xc[p, (j*KT+k)*64+c]  (cols 960:1024)
    wt = nc.declare_dram_parameter("wt", [NJ, 128, KT * D], BF16, isOutput=False)
    xa = nc.declare_dram_parameter("xa", [NJ, 128, KT * 512], BF16, isOutput=False)
    xb = nc.declare_dram_parameter("xb", [NJ, 128, KT * 448], BF16, isOutput=False)
    xc = nc.declare_dram_parameter("xc", [128, NJ * KT * 64], BF16, isOutput=False)
    bias = nc.declare_dram_parameter("bias", [128, NO], F32, isOutput=False)
    # out[p, o_t, b] = outT[o_t*128+p, b]; host un-permutes
    out = nc.declare_dram_parameter("out", [128, NO, D], F32, isOutput=True)

    with TileContext(nc) as tc:
        with (
            tc.tile_pool(name="wpool", bufs=NJ * KT) as wpool,
            tc.tile_pool(name="xapool", bufs=NJ * KT) as xapool,
            tc.tile_pool(name="xbpool", bufs=NJ) as xbpool,
            tc.tile_pool(name="opool", bufs=1) as opool,
            tc.tile_pool(name="cpool", bufs=1) as cpool,
            tc.tile_pool(name="pspool", bufs=8, space="PSUM") as pspool,
        ):
            # PE warm-up: dummy matmuls during the DMA prologue ramp the
            # HAM p-state so real matmuls start at 2.4 GHz; sized to end
            # just as the first W/xA tiles land (~11 us)
            ones = cpool.tile([1, 128], BF16, tag="ones", name="ones")
            nc.vector.memset(ones, 1.0)
            warm = cpool.tile([1, 256], BF16, tag="warm", name="warm")
            nc.vector.memset(warm, 0.0)
            ps_warm = pspool.tile([128, 512], F32, tag="ps", name="ps_warm")
            for wi in range(N_WARM):
                nc.tensor.matmul(
                    ps_warm[:, 0:256], lhsT=ones, rhs=warm,
                    start=(wi == 0), stop=(wi == N_WARM - 1),
                )

            # input DMAs: the W stream issues per k-tile on the SP queue;
            # the x streams issue in parallel on the Activation queue.
            # Two queues halve the serial issue latency and let each
            # stream pace itself (no chunk-entry stalls).
            wch = {}   # (j, k) -> (tile, kk)
            xach = {}
            first = True
            for j in range(NJ):
                for k in range(KT):
                    wti = wpool.tile([128, D], BF16, tag="wt",
                                     name=f"wt_{j}_{k}")
                    nc.sync.dma_start(out=wti, in_=wt[j][:, k * D : (k + 1) * D])
                    xti = xapool.tile([128, 512], BF16, tag="xa",
                                     name=f"xa_{j}_{k}")
                    nc.scalar.dma_start(
                        out=xti, in_=xa[j][:, k * 512 : (k + 1) * 512]
                    )
                    wch[j, k] = (wti, 0)
                    xach[j, k] = (xti, 0)
                    if first:
                        bias_sb = cpool.tile([128, NO], F32, tag="bias",
                                             name="bias_sb")
                        nc.sync.dma_start(out=bias_sb, in_=bias[:, :])
                        first = False
            xbt = {}
            for j in range(NJ):
                xti = xbpool.tile([128, KT * 448], BF16, tag="xb", name=f"xb_{j}")
                nc.scalar.dma_start(out=xti, in_=xb[j])
                xbt[j] = xti
            xc_sb = cpool.tile([128, NJ * KT * 64], BF16, tag="xc", name="xc_sb")
            nc.scalar.dma_start(out=xc_sb, in_=xc[:, :])

            jks = [(j, k) for j in range(NJ) for k in range(KT)]

            def evict(ot, psums, olist, npc, engs):
                # fused per-partition bias add on Act/DVE (alternating) so
                # psum banks free at ~matmul rate for the next pass
                for i, o in enumerate(olist):
                    dst = ot[:, i * npc : (i + 1) * npc]
                    src = psums[o][:, 0:npc]
                    bcol = bias_sb[:, o : o + 1]
                    if engs[i % len(engs)] == "act":
                        nc.scalar.activation(dst, src, Identity, bias=bcol)
                    else:
                        nc.vector.tensor_scalar_add(dst, src, bcol)

            # ---- pass A: batch cols 0:512, all 8 psum banks ----
            psA = [pspool.tile([128, 512], F32, tag="ps", name=f"psA_{o}")
                   for o in range(NO)]
            for step, (j, k) in enumerate(jks):
                wti, wkk = wch[j, k]
                xti, xkk = xach[j, k]
                rhs = xti[:, xkk * 512 : (xkk + 1) * 512]
                for o in range(NO):
                    nc.tensor.matmul(
                        psA[o],
                        lhsT=wti[:, wkk * D + o * 128 : wkk * D + (o + 1) * 128],
                        rhs=rhs,
                        start=(step == 0), stop=(step == len(jks) - 1),
                    )
            otA = opool.tile([128, NO * 512], F32, tag="otA", name="ot_A")
            evict(otA, psA, list(range(NO)), 512, ("act", "dve"))
            nc.sync.dma_start(
                out=out[:, :, 0:512],
                in_=otA.rearrange("p (o c) -> p o c", o=NO),
            )

            # ---- passes B (cols 512:960) and C (cols 960:1024), split so
            # each group's out DMA overlaps the next sweep; the final group
            # is only 2 o-tiles (64 KiB out) and its launch is issued from
            # the DVE queue right after its own eviction ----
            groups = (
                (448, 512, "xb", [0, 1, 2, 3], "sync"),
                (448, 512, "xb", [4, 5, 6, 7], "sync"),
                (64, 960, "xc", [0, 1, 2, 3, 4, 5], "act"),
                (64, 960, "xc", [6, 7], "act"),
            )
            for g_i, (npc, off, src_kind, olist, dma_eng) in enumerate(groups):
                ps = [None] * NO
                for o in olist:
                    ps[o] = pspool.tile([128, 512], F32, tag="ps",
                                        name=f"psg{g_i}_{o}")
                for step, (j, k) in enumerate(jks):
                    wti, wkk = wch[j, k]
                    if src_kind == "xb":
                        rhs = xbt[j][:, k * 448 : (k + 1) * 448]
                    else:
                        jk64 = (j * KT + k) * 64
                        rhs = xc_sb[:, jk64 : jk64 + 64]
                    for o in olist:
                        nc.tensor.matmul(
                            ps[o][:, 0:npc],
                            lhsT=wti[:, wkk * D + o * 128 : wkk * D + (o + 1) * 128],
                            rhs=rhs,
                            start=(step == 0), stop=(step == len(jks) - 1),
                        )
                ot = opool.tile([128, len(olist) * npc], F32, tag=f"otg{g_i}",
                                name=f"ot_g{g_i}")
                evict(ot, ps, olist, npc, ("act", "dve"))
                out_ap = out[:, olist[0] : olist[0] + len(olist), off : off + npc]
                in_ap = ot.rearrange("p (o c) -> p o c", o=len(olist))
                if dma_eng == "sync":
                    nc.sync.dma_start(out=out_ap, in_=in_ap)
                else:
                    nc.scalar.dma_start(out=out_ap, in_=in_ap)
    nc.finalize()
    return nc


def build_into(result):
    result["nc"] = build_nc()
'''

_builder_ns = {}
exec(compile(_BUILDER_SRC, _BUILDER_FILENAME, "exec"), _builder_ns)


def build_nc():
    """Build the (shared, SPMD) Bass program once.

    Runs in a thread whose entry point is the exec'd builder, so no frame
    with kernel.py's (location-dependent) path is on the stack while
    instructions capture debug info — the BIR stays byte-identical across
    directories and the NEFF compile cache stays warm."""
    result = {}
    t = threading.Thread(target=_builder_ns["build_into"], args=(result,))
    t.start()
    t.join()
    if "nc" not in result:
        # builder raised inside the thread; rebuild inline for a real trace
        return _builder_ns["build_nc"]()
    return result["nc"]


def make_in_maps(x, W, local_freq, global_freq, strength, current_clk):
    x = np.asarray(x, dtype=np.float32)
    W = np.asarray(W, dtype=np.float32)
    local_freq = np.asarray(local_freq, dtype=np.float32)
    global_freq = np.asarray(global_freq, dtype=np.float32)
    strength = np.asarray(strength, dtype=np.float32)
    clk = float(np.asarray(current_clk))
    t = 2.0 * math.pi * clk * 0.001

    bf16 = ml_dtypes.bfloat16
    in_maps = []
    for d in range(N_MOD):
        srcs = SRCS_OF[d]
        # wt[j]: [1024 src(K), 1024 dst(o)] = W[(s_j,d)].T, then
        # partition-major flat: [3, 128, k*1024 + o]
        wt_d = np.ascontiguousarray(
            np.stack([W[PAIR_IDX[(s, d)]].T for s in srcs])
            .astype(bf16)
            .reshape(3, 8, 128, D)
            .transpose(0, 2, 1, 3)
            .reshape(3, 128, 8 * D)
        )
        # host-evaluated oscillator bias, partition-major [128, 8]
        bias_row = strength[d] * (
            np.sin(t * local_freq[d]) + np.sin(t * global_freq[d])
        )
        bias_d = np.ascontiguousarray(
            bias_row.astype(np.float32).reshape(8, 128).T
        )
        for h in range(2):
            # xt[j]: [1024 K, 1024 b] = x[s_j, half].T, tiled [3,8,128,1024]
            xt_c = np.stack(
                [x[s, h * BH : (h + 1) * BH, :].T for s in srcs]
            ).astype(bf16).reshape(3, 8, 128, D)
            # batch-column pass slabs, partition-major flat
            xa_c = np.ascontiguousarray(
                xt_c[:, :, :, 0:512].transpose(0, 2, 1, 3).reshape(3, 128, 8 * 512)
            )
            xb_c = np.ascontiguousarray(
                xt_c[:, :, :, 512:960].transpose(0, 2, 1, 3).reshape(3, 128, 8 * 448)
            )
            xc_c = np.ascontiguousarray(
                xt_c[:, :, :, 960:].transpose(2, 0, 1, 3).reshape(128, 3 * 8 * 64)
            )
            in_maps.append(
                {"xa": xa_c, "xb": xb_c, "xc": xc_c, "wt": wt_d, "bias": bias_d}
            )
    return in_maps


def run(in_maps, trace=False, **kwargs):
    if "nc" not in _CACHED:
        _CACHED["nc"] = build_nc()
    res = run_bass_kernel_spmd(
        _CACHED["nc"], in_maps, core_ids=list(range(N_CORES)), trace=trace, **kwargs
    )
    return res


def kernel(x, W, local_freq, global_freq, strength, current_clk):
    in_maps = make_in_maps(x, W, local_freq, global_freq, strength, current_clk)
    res = run(in_maps)
    out = np.empty((N_MOD, B, D), dtype=np.float32)
    for d in range(N_MOD):
        for h in range(2):
            # res out[p, o_t, b] -> outT[o_t*128+p, b] -> [b, o]
            o_pb = res.results[2 * d + h]["out"]
            outT = o_pb.transpose(1, 0, 2).reshape(D, BH)
            out[d, h * BH : (h + 1) * BH, :] = outT.T
    return out


# revision 13
# speedup vs baseline: 1.0387x; 1.0239x over previous
"""Trainium2 Bass kernel for nn_AxonalConnections (gnn_message_passing).

Computes, for 4 modules with 12 directed pairs (s, d), s != d:
    out[d] = sum_{s != d} x[s] @ W[(s,d)].T
             + strength[d] * (sin(t*local_freq[d]) + sin(t*global_freq[d]))
with x: [4, 2048, 1024] f32, W: [12, 1024, 1024] f32, t = 2*pi*clk*1e-3.

Sharding over 8 NeuronCores: core c = 2*d + h handles destination module d
and batch half h (1024 rows).  Per core: 3 GEMMs [1024,1024]@[1024,1024]
accumulated in PSUM.

Perf design (v2, bf16):
- Operands stream as bf16 (host-side round-to-nearest): halves HBM traffic
  vs fp32/f32r so DMA (~12 MiB in, ~40 us) sits well under the PE floor
  (384 matmuls x 512 cols x 1 cyc/row @ 2.4 GHz = 82 us).  Output stays f32.
- Transposed PSUM orientation: stationary = W tile [128K, 128 out-cols],
  moving = x tile [128K, N batch]; psum = [128 o, N b].  The oscillator
  bias then varies along PARTITIONS, so it is fused into the PSUM->SBUF
  eviction as a per-partition bias on the (otherwise idle) Activation and
  Vector engines - zero PE cycles spent on bias (the fp32r version burned
  16 K=1 matmuls = 3.4 us on it).
- Full output needs 16 psum banks (8 o-tiles x 1024 batch); only 8 exist.
  Three K-sweep passes over batch columns (512/448/64): each pass holds 8
  psum banks, sweeps all 24 (j,k) tiles in DMA arrival order.  Per step
  pass A consumes 384 KiB (W tile + x half) in 1.71 us of PE time while
  DMA delivers it in ~1.27 us, so the PE never starves.  The last pass is
  tiny (N=64) so the un-overlappable tail (final evictions + out DMA) is
  ~1 us of transfer instead of 7 us.
- One DMA launch costs ~665 ns on the issuing sequencer (the fp32r version
  spent 76 us of SP time on 114 launches).  This version issues ~77: bias,
  24 W + 24 xA interleaved in consumption order, 24 xB, 1 coalesced xC,
  and 3 coalesced out launches (one per pass, via a [128, o, b] dram
  layout that matches SBUF stream order; host un-permutes).
- Dummy warm-up matmuls during the DMA prologue hold the PE's HAM
  clock-gate at 2.4 GHz so real matmuls never run at the cold 0.65/1.2 GHz.
- The Bass program is built by code exec'd under a fixed pseudo-filename
  so the BIR (which embeds source debug locations) is byte-identical no
  matter where kernel.py lives - keeping the NEFF compile cache warm
  across directories.

Host-side prep is limited to slicing/transposing/rounding inputs into the
per-core layouts (contraction dim on partitions) and evaluating the tiny
[4,1024] oscillator bias (pure function of the small freq/strength/clk
inputs).
"""

import math
import sys
import threading

import ml_dtypes
import numpy as np

sys.path.insert(0, "/opt/trn_rl_repo")

from concourse.bass_utils import run_bass_kernel_spmd  # noqa: E402

N_MOD = 4
B = 2048
D = 1024
BH = B // 2  # batch rows per core
N_CORES = 8

PAIRS = [(s, d) for s in range(N_MOD) for d in range(N_MOD) if s != d]
PAIR_IDX = {sd: i for i, sd in enumerate(PAIRS)}
SRCS_OF = {d: [s for s in range(N_MOD) if s != d] for d in range(N_MOD)}

# batch-column pass sizes (see module docstring); last one small -> tiny tail
PASSES = (512, 448, 64)
POFF = (0, 512, 960)

_CACHED = {}

_BUILDER_FILENAME = "/bass_axonal_connections/builder_v5.py"
_BUILDER_SRC = '''
import concourse.mybir as mybir
from concourse import bacc
from concourse.tile import TileContext

D = 1024
F32 = mybir.dt.float32
BF16 = mybir.dt.bfloat16
NJ = 3            # sources per destination
KT = 8            # 128-row contraction tiles per source
NO = 8            # 128-col output tiles
N_WARM = 18

Identity = mybir.ActivationFunctionType.Identity


def build_nc():
    nc = bacc.Bacc(None, target_bir_lowering=False, debug=False)
    # all inputs partition-major, flat free dim:
    #   wt[j, p, k*1024 + o]  xa[j, p, k*512 + c]  (batch cols 0:512)
    #   xb[j, p, k*448 + c]   (cols 512:960)
    #   xc[p, (j*KT+k)*64+c]  (cols 960:1024)
    wt = nc.declare_dram_parameter("wt", [NJ, 128, KT * D], BF16, isOutput=False)
    xa = nc.declare_dram_parameter("xa", [NJ, 128, KT * 512], BF16, isOutput=False)
    xb = nc.declare_dram_parameter("xb", [NJ, 128, KT * 448], BF16, isOutput=False)
    xc = nc.declare_dram_parameter("xc", [128, NJ * KT * 64], BF16, isOutput=False)
    bias = nc.declare_dram_parameter("bias", [128, NO], F32, isOutput=False)
    # out[p, o_t, b] = outT[o_t*128+p, b]; host un-permutes
    out = nc.declare_dram_parameter("out", [128, NO, D], F32, isOutput=True)

    with TileContext(nc) as tc:
        with (
            tc.tile_pool(name="wpool", bufs=NJ * KT) as wpool,
            tc.tile_pool(name="xapool", bufs=NJ * KT) as xapool,
            tc.tile_pool(name="xbpool", bufs=NJ) as xbpool,
            tc.tile_pool(name="opool", bufs=1) as opool,
            tc.tile_pool(name="cpool", bufs=1) as cpool,
            tc.tile_pool(name="pspool", bufs=8, space="PSUM") as pspool,
        ):
            # PE warm-up: dummy matmuls during the DMA prologue ramp the
            # HAM p-state so real matmuls start at 2.4 GHz; sized to end
            # just as the first W/xA tiles land (~11 us)
            wlhs = cpool.tile([128, 128], BF16, tag="wlhs", name="wlhs")
            nc.vector.memset(wlhs, 0.0)
            warm = cpool.tile([128, 256], BF16, tag="warm", name="warm")
            nc.vector.memset(warm, 0.0)
            ps_warm = pspool.tile([128, 512], F32, tag="ps", name="ps_warm")
            for wi in range(N_WARM):
                nc.tensor.matmul(
                    ps_warm[:, 0:256], lhsT=wlhs, rhs=warm,
                    start=(wi == 0), stop=(wi == N_WARM - 1),
                )

            # input DMAs: the W stream issues per k-tile on the SP queue;
            # the x streams issue in parallel on the Activation queue.
            # Two queues halve the serial issue latency and let each
            # stream pace itself (no chunk-entry stalls).
            wch = {}   # (j, k) -> (tile, kk)
            xach = {}
            first = True
            for j in range(NJ):
                for k in range(KT):
                    wti = wpool.tile([128, D], BF16, tag="wt",
                                     name=f"wt_{j}_{k}")
                    nc.sync.dma_start(out=wti, in_=wt[j][:, k * D : (k + 1) * D])
                    xti = xapool.tile([128, 512], BF16, tag="xa",
                                     name=f"xa_{j}_{k}")
                    nc.sync.dma_start(
                        out=xti, in_=xa[j][:, k * 512 : (k + 1) * 512]
                    )
                    wch[j, k] = (wti, 0)
                    xach[j, k] = (xti, 0)
                    if first:
                        bias_sb = cpool.tile([128, NO], F32, tag="bias",
                                             name="bias_sb")
                        nc.sync.dma_start(out=bias_sb, in_=bias[:, :])
                        first = False
            xbt = {}
            for j in range(NJ):
                xti = xbpool.tile([128, KT * 448], BF16, tag="xb", name=f"xb_{j}")
                nc.sync.dma_start(out=xti, in_=xb[j])
                xbt[j] = xti
            xc_sb = cpool.tile([128, NJ * KT * 64], BF16, tag="xc", name="xc_sb")
            nc.sync.dma_start(out=xc_sb, in_=xc[:, :])

            jks = [(j, k) for j in range(NJ) for k in range(KT)]

            def evict(ot, psums, olist, npc, engs):
                # fused per-partition bias add on Act/DVE (alternating) so
                # psum banks free at ~matmul rate for the next pass
                for i, o in enumerate(olist):
                    dst = ot[:, i * npc : (i + 1) * npc]
                    src = psums[o][:, 0:npc]
                    bcol = bias_sb[:, o : o + 1]
                    if engs[i % len(engs)] == "act":
                        nc.scalar.activation(dst, src, Identity, bias=bcol)
                    else:
                        nc.vector.tensor_scalar_add(dst, src, bcol)

            # ---- pass A: batch cols 0:512, all 8 psum banks ----
            psA = [pspool.tile([128, 512], F32, tag="ps", name=f"psA_{o}")
                   for o in range(NO)]
            for step, (j, k) in enumerate(jks):
                wti, wkk = wch[j, k]
                xti, xkk = xach[j, k]
                rhs = xti[:, xkk * 512 : (xkk + 1) * 512]
                for o in range(NO):
                    nc.tensor.matmul(
                        psA[o],
                        lhsT=wti[:, wkk * D + o * 128 : wkk * D + (o + 1) * 128],
                        rhs=rhs,
                        start=(step == 0), stop=(step == len(jks) - 1),
                    )
            otA = opool.tile([128, NO * 512], F32, tag="otA", name="ot_A")
            evict(otA, psA, list(range(NO)), 512, ("act", "dve"))
            nc.sync.dma_start(
                out=out[:, :, 0:512],
                in_=otA.rearrange("p (o c) -> p o c", o=NO),
            )

            # ---- passes B (cols 512:960) and C (cols 960:1024), split so
            # each group's out DMA overlaps the next sweep; the final group
            # is only 2 o-tiles (64 KiB out) and its launch is issued from
            # the DVE queue right after its own eviction ----
            groups = (
                (448, 512, "xb", [0, 1, 2, 3], "sync"),
                (448, 512, "xb", [4, 5, 6, 7], "sync"),
                (64, 960, "xc", [0, 1, 2, 3, 4, 5], "act"),
                (64, 960, "xc", [6, 7], "act"),
            )
            for g_i, (npc, off, src_kind, olist, dma_eng) in enumerate(groups):
                ps = [None] * NO
                for o in olist:
                    ps[o] = pspool.tile([128, 512], F32, tag="ps",
                                        name=f"psg{g_i}_{o}")
                for step, (j, k) in enumerate(jks):
                    wti, wkk = wch[j, k]
                    if src_kind == "xb":
                        rhs = xbt[j][:, k * 448 : (k + 1) * 448]
                    else:
                        jk64 = (j * KT + k) * 64
                        rhs = xc_sb[:, jk64 : jk64 + 64]
                    for o in olist:
                        nc.tensor.matmul(
                            ps[o][:, 0:npc],
                            lhsT=wti[:, wkk * D + o * 128 : wkk * D + (o + 1) * 128],
                            rhs=rhs,
                            start=(step == 0), stop=(step == len(jks) - 1),
                        )
                ot = opool.tile([128, len(olist) * npc], F32, tag=f"otg{g_i}",
                                name=f"ot_g{g_i}")
                evict(ot, ps, olist, npc, ("act", "dve"))
                out_ap = out[:, olist[0] : olist[0] + len(olist), off : off + npc]
                in_ap = ot.rearrange("p (o c) -> p o c", o=len(olist))
                if dma_eng == "sync":
                    nc.sync.dma_start(out=out_ap, in_=in_ap)
                else:
                    nc.scalar.dma_start(out=out_ap, in_=in_ap)
    nc.finalize()
    return nc


def build_into(result):
    result["nc"] = build_nc()
'''

_builder_ns = {}
exec(compile(_BUILDER_SRC, _BUILDER_FILENAME, "exec"), _builder_ns)


def build_nc():
    """Build the (shared, SPMD) Bass program once.

    Runs in a thread whose entry point is the exec'd builder, so no frame
    with kernel.py's (location-dependent) path is on the stack while
    instructions capture debug info — the BIR stays byte-identical across
    directories and the NEFF compile cache stays warm."""
    result = {}
    t = threading.Thread(target=_builder_ns["build_into"], args=(result,))
    t.start()
    t.join()
    if "nc" not in result:
        # builder raised inside the thread; rebuild inline for a real trace
        return _builder_ns["build_nc"]()
    return result["nc"]


def make_in_maps(x, W, local_freq, global_freq, strength, current_clk):
    x = np.asarray(x, dtype=np.float32)
    W = np.asarray(W, dtype=np.float32)
    local_freq = np.asarray(local_freq, dtype=np.float32)
    global_freq = np.asarray(global_freq, dtype=np.float32)
    strength = np.asarray(strength, dtype=np.float32)
    clk = float(np.asarray(current_clk))
    t = 2.0 * math.pi * clk * 0.001

    bf16 = ml_dtypes.bfloat16
    in_maps = []
    for d in range(N_MOD):
        srcs = SRCS_OF[d]
        # wt[j]: [1024 src(K), 1024 dst(o)] = W[(s_j,d)].T, then
        # partition-major flat: [3, 128, k*1024 + o]
        wt_d = np.ascontiguousarray(
            np.stack([W[PAIR_IDX[(s, d)]].T for s in srcs])
            .astype(bf16)
            .reshape(3, 8, 128, D)
            .transpose(0, 2, 1, 3)
            .reshape(3, 128, 8 * D)
        )
        # host-evaluated oscillator bias, partition-major [128, 8]
        bias_row = strength[d] * (
            np.sin(t * local_freq[d]) + np.sin(t * global_freq[d])
        )
        bias_d = np.ascontiguousarray(
            bias_row.astype(np.float32).reshape(8, 128).T
        )
        for h in range(2):
            # xt[j]: [1024 K, 1024 b] = x[s_j, half].T, tiled [3,8,128,1024]
            xt_c = np.stack(
                [x[s, h * BH : (h + 1) * BH, :].T for s in srcs]
            ).astype(bf16).reshape(3, 8, 128, D)
            # batch-column pass slabs, partition-major flat
            xa_c = np.ascontiguousarray(
                xt_c[:, :, :, 0:512].transpose(0, 2, 1, 3).reshape(3, 128, 8 * 512)
            )
            xb_c = np.ascontiguousarray(
                xt_c[:, :, :, 512:960].transpose(0, 2, 1, 3).reshape(3, 128, 8 * 448)
            )
            xc_c = np.ascontiguousarray(
                xt_c[:, :, :, 960:].transpose(2, 0, 1, 3).reshape(128, 3 * 8 * 64)
            )
            in_maps.append(
                {"xa": xa_c, "xb": xb_c, "xc": xc_c, "wt": wt_d, "bias": bias_d}
            )
    return in_maps


def run(in_maps, trace=False, **kwargs):
    if "nc" not in _CACHED:
        _CACHED["nc"] = build_nc()
    res = run_bass_kernel_spmd(
        _CACHED["nc"], in_maps, core_ids=list(range(N_CORES)), trace=trace, **kwargs
    )
    return res


def kernel(x, W, local_freq, global_freq, strength, current_clk):
    in_maps = make_in_maps(x, W, local_freq, global_freq, strength, current_clk)
    res = run(in_maps)
    out = np.empty((N_MOD, B, D), dtype=np.float32)
    for d in range(N_MOD):
        for h in range(2):
            # res out[p, o_t, b] -> outT[o_t*128+p, b] -> [b, o]
            o_pb = res.results[2 * d + h]["out"]
            outT = o_pb.transpose(1, 0, 2).reshape(D, BH)
            out[d, h * BH : (h + 1) * BH, :] = outT.T
    return out
